# revision 3
# baseline (speedup 1.0000x reference)
"""Trainium2 kernel for the quantum-circuit AENN problem.

The reference applies a fixed 10-qubit variational circuit (186 params) to
each normalized input row, takes |amp|^2, rescales by norm^2, and applies a
Dense layer.  The circuit is LINEAR in the state, so it is a fixed 1024x1024
complex unitary U, and the normalization cancels exactly:

    norm^2 * |U (x/norm)|^2 = |U x|^2

so:  out = ((X @ Ur^T)^2 + (X @ Ui^T)^2) @ kernel + bias

Host side: build U from the 186 weights (tiny), pack W = [Ur^T | Ui^T] in
bf16, pre-transpose X.  Device side (pure data parallelism, batch sharded
512 rows/core, no collectives): Y^T = W^T-blocks x X^T via TensorE (bf16,
fp32 accumulate), probs^T = Yr^2 + Yi^2 (ScalarE squares + VectorE add),
out^T = kernel^T @ probs^T (TensorE), bias add, transpose, DMA out.
"""

import os
import numpy as np
import ml_dtypes

NUM_QUBITS = 10
LAYER_DEPTH = 4
DIM = 2 ** NUM_QUBITS            # 1024
BATCH = 4096
NUM_OUTPUT = 10
SIZE_ROT = (LAYER_DEPTH + 1) * NUM_QUBITS * 3   # 150
N_CORES = 8
ROWS = BATCH // N_CORES          # 512 rows per core
KT = DIM // 128                  # 8 k-tiles of 128 along the feature dim
AT = DIM // 128                  # 8 amplitude tile-pairs (Re,Im) of 128

_BF16 = ml_dtypes.bfloat16
_CACHE = {}
LAST_RESULTS = None  # BassKernelResults of the most recent run (for test.py)


# ----------------------------------------------------------------------------
# Host: build the circuit unitary U (amp = U @ psi)
# ----------------------------------------------------------------------------
def _build_unitary(qw: np.ndarray) -> np.ndarray:
    qw = np.asarray(qw, dtype=np.float64)
    rotations = qw[:SIZE_ROT].reshape(LAYER_DEPTH + 1, NUM_QUBITS, 3)
    rxx = qw[SIZE_ROT:].reshape(LAYER_DEPTH, NUM_QUBITS - 1)

    # Columns of the identity, qubit axes unpacked: shape (2,)*10 + (DIM,)
    M = np.eye(DIM, dtype=np.complex128).reshape((2,) * NUM_QUBITS + (DIM,))

    def apply_r(M, theta, phi, alpha, j):
        sa = np.sin(alpha)
        nx = sa * np.cos(phi)
        ny = sa * np.sin(phi)
        nz = np.cos(alpha)
        ct = np.cos(theta)
        mist = -1j * np.sin(theta)
        U2 = np.array([
            [ct + mist * nz, mist * (nx - 1j * ny)],
            [mist * (nx + 1j * ny), ct - mist * nz],
        ], dtype=np.complex128)
        M = np.tensordot(U2, M, axes=[[1], [j]])
        return np.moveaxis(M, 0, j)

    for k in range(LAYER_DEPTH):
        for j in range(NUM_QUBITS):
            M = apply_r(M, rotations[k, j, 0], rotations[k, j, 1],
                        rotations[k, j, 2], j)
        for j in range(NUM_QUBITS - 1):
            flipped = np.flip(M, axis=(j, j + 1))
            M = np.cos(rxx[k, j]) * M + (-1j * np.sin(rxx[k, j])) * flipped
    for j in range(NUM_QUBITS):
        M = apply_r(M, rotations[LAYER_DEPTH, j, 0],
                    rotations[LAYER_DEPTH, j, 1],
                    rotations[LAYER_DEPTH, j, 2], j)
    return M.reshape(DIM, DIM)   # U with amp = U @ psi


# ----------------------------------------------------------------------------
# Device graph (built once, cached)
# ----------------------------------------------------------------------------
def _build_graph():
    from concourse import bacc
    import concourse.mybir as mybir
    import concourse.tile as tile
    from concourse.bass import ts
    from concourse.masks import make_identity

    f32 = mybir.dt.float32
    bf16 = mybir.dt.bfloat16

    nc = bacc.Bacc("TRN2", target_bir_lowering=False, debug=False,
                   num_devices=N_CORES)

    xt_d = nc.dram_tensor("xt", [KT, 128, ROWS], bf16, kind="ExternalInput")
    w_d = nc.dram_tensor("w", [AT, KT, 128, 256], bf16, kind="ExternalInput")
    kc_d = nc.dram_tensor("kc", [128, AT, NUM_OUTPUT], bf16, kind="ExternalInput")
    bias_d = nc.dram_tensor("bias", [NUM_OUTPUT, 1], f32, kind="ExternalInput")
    out_d = nc.dram_tensor("out", [ROWS, NUM_OUTPUT], f32, kind="ExternalOutput")

    with tile.TileContext(nc) as tc:
        with (
            tc.tile_pool(name="xtp", bufs=KT) as xtp,
            tc.tile_pool(name="wp", bufs=2 * KT) as wp,
            tc.tile_pool(name="cst", bufs=1) as cst,
            tc.tile_pool(name="sq", bufs=2) as sqp,
            tc.tile_pool(name="pb", bufs=2) as pbp,
            tc.tile_pool(name="psmm", bufs=2, space="PSUM") as psmm,
            tc.tile_pool(name="psout", bufs=1, space="PSUM") as psout,
            tc.tile_pool(name="pstr", bufs=2, space="PSUM") as pstr,
        ):
            # Resident inputs
            xt_tiles = []
            for k in range(KT):
                xk = xtp.tile([128, ROWS], bf16)
                nc.sync.dma_start(out=xk[:], in_=xt_d[k])
                xt_tiles.append(xk)
            k_sb = cst.tile([128, AT, NUM_OUTPUT], bf16)
            nc.sync.dma_start(out=k_sb[:], in_=kc_d[:])
            bias_sb = cst.tile([NUM_OUTPUT, 1], f32)
            nc.sync.dma_start(out=bias_sb[:], in_=bias_d[:])
            ident = cst.tile([128, 128], f32)
            make_identity(nc, ident[:])

            outT_ps = psout.tile([NUM_OUTPUT, ROWS], f32)

            for t in range(AT):
                wts = []
                for k in range(KT):
                    wt = wp.tile([128, 256], bf16)
                    nc.sync.dma_start(out=wt[:], in_=w_d[t, k])
                    wts.append(wt)
                ps_re = psmm.tile([128, ROWS], f32)
                for k in range(KT):
                    nc.tensor.matmul(ps_re[:], wts[k][:, 0:128], xt_tiles[k][:],
                                     start=(k == 0), stop=(k == KT - 1))
                ps_im = psmm.tile([128, ROWS], f32)
                for k in range(KT):
                    nc.tensor.matmul(ps_im[:], wts[k][:, 128:256], xt_tiles[k][:],
                                     start=(k == 0), stop=(k == KT - 1))
                sq_re = sqp.tile([128, ROWS], f32)
                nc.scalar.square(sq_re[:], ps_re[:])
                sq_im = sqp.tile([128, ROWS], f32)
                nc.scalar.square(sq_im[:], ps_im[:])
                p_t = pbp.tile([128, ROWS], bf16)
                nc.vector.tensor_add(p_t[:], sq_re[:], sq_im[:])
                nc.tensor.matmul(outT_ps[:], k_sb[:, t, :], p_t[:],
                                 start=(t == 0), stop=(t == AT - 1),
                                 skip_group_check=True)

            # Epilogue: bias add, transpose [10, 512] -> [512, 10], store.
            outT_sb = cst.tile([NUM_OUTPUT, ROWS], f32)
            nc.scalar.activation(outT_sb[:], outT_ps[:],
                                 mybir.ActivationFunctionType.Identity,
                                 bias=bias_sb[:], scale=1.0)
            out_sb = cst.tile([128, ROWS // 128, NUM_OUTPUT], f32)
            for rt in range(ROWS // 128):
                tr_ps = pstr.tile([128, NUM_OUTPUT], f32)
                nc.tensor.transpose(tr_ps[:], outT_sb[:, ts(rt, 128)],
                                    ident[:NUM_OUTPUT, :NUM_OUTPUT])
                nc.vector.tensor_copy(out_sb[:, rt, :], tr_ps[:])
            out_view = out_d.ap().rearrange("(rt p) o -> p rt o", p=128)
            nc.sync.dma_start(out=out_view, in_=out_sb[:])

    nc.compile()
    return nc


def _ensure_ntff_hook():
    """The trace path does `from antenv.axon_hooks import ...`; some images
    lack that optional module.  Provide it (wired to the axon PJRT .so when
    available) so BASS_TRACE=1 profiles instead of crashing."""
    try:
        import antenv.axon_hooks  # noqa: F401
        return
    except ImportError:
        pass
    import sys
    import types
    try:
        import antenv
    except ImportError:
        return
    mod = types.ModuleType("antenv.axon_hooks")
    state = {"hook": None}
    mod.set_axon_ntff_profile_hook = lambda h: state.__setitem__("hook", h)
    mod.get_axon_ntff_profile_hook = lambda: state["hook"]
    sys.modules["antenv.axon_hooks"] = mod
    antenv.axon_hooks = mod
    try:
        from trn_agent_boot.trn_boot import _ntff_profile_via_ctypes
        so_path = "/opt/axon/libaxon_pjrt.so"
        if os.path.exists(so_path):
            hook = _ntff_profile_via_ctypes(so_path)
            if hook is not None:
                mod.set_axon_ntff_profile_hook(hook)
    except Exception:
        pass


# ----------------------------------------------------------------------------
# Entry point
# ----------------------------------------------------------------------------
def kernel(x, quantum_weights, kernel, bias):
    global LAST_RESULTS
    _ensure_ntff_hook()
    from concourse.bass_utils import run_bass_kernel_spmd

    x = np.asarray(x, dtype=np.float32)
    qw = np.asarray(quantum_weights, dtype=np.float32)
    kmat = np.asarray(kernel, dtype=np.float32)
    bvec = np.asarray(bias, dtype=np.float32)

    U = _build_unitary(qw)
    Ur = np.ascontiguousarray(U.real, dtype=np.float64)
    Ui = np.ascontiguousarray(U.imag, dtype=np.float64)
    # w[t, k, p, j]: j<128 -> Ur[128t+j, 128k+p]; j>=128 -> Ui[128t+j-128, 128k+p]
    Ur4 = Ur.reshape(AT, 128, KT, 128).transpose(0, 2, 3, 1)  # [t, k, p, j]
    Ui4 = Ui.reshape(AT, 128, KT, 128).transpose(0, 2, 3, 1)
    w4 = np.concatenate([Ur4, Ui4], axis=3).astype(_BF16)      # [AT, KT, 128, 256]

    kc = np.ascontiguousarray(
        kmat.reshape(AT, 128, NUM_OUTPUT).transpose(1, 0, 2)).astype(_BF16)
    bias2 = np.ascontiguousarray(bvec.reshape(NUM_OUTPUT, 1))

    if "nc" not in _CACHE:
        _CACHE["nc"] = _build_graph()
    nc = _CACHE["nc"]

    in_maps = []
    for c in range(N_CORES):
        xs = x[c * ROWS:(c + 1) * ROWS]                        # [512, 1024]
        xt = np.ascontiguousarray(xs.T).astype(_BF16).reshape(KT, 128, ROWS)
        in_maps.append({"xt": xt, "w": w4, "kc": kc, "bias": bias2})

    res = run_bass_kernel_spmd(nc, in_maps, core_ids=list(range(N_CORES)))
    LAST_RESULTS = res
    return np.concatenate([r["out"] for r in res.results], axis=0)


# revision 5
# speedup vs baseline: 1.2956x; 1.2956x over previous
"""Trainium2 kernel for the quantum-circuit AENN problem.

The reference applies a fixed 10-qubit variational circuit (186 params) to
each normalized input row, takes |amp|^2, rescales by norm^2, and applies a
Dense layer.  The circuit is LINEAR in the state, so it is a fixed 1024x1024
complex unitary U, and the normalization cancels exactly:

    norm^2 * |U (x/norm)|^2 = |U x|^2

so:  out = ((X @ Ur^T)^2 + (X @ Ui^T)^2) @ kernel + bias

Host side: build U from the 186 weights (tiny), pack W = [Ur^T | Ui^T] in
bf16, pre-transpose X.  Device side (pure data parallelism, batch sharded
512 rows/core, no collectives): Y^T = W^T-blocks x X^T via TensorE (bf16,
fp32 accumulate), probs^T = Yr^2 + Yi^2 (ScalarE squares + VectorE add),
out^T = kernel^T @ probs^T (TensorE), bias add, transpose, DMA out.
"""

import os
import numpy as np
import ml_dtypes

NUM_QUBITS = 10
LAYER_DEPTH = 4
DIM = 2 ** NUM_QUBITS            # 1024
BATCH = 4096
NUM_OUTPUT = 10
SIZE_ROT = (LAYER_DEPTH + 1) * NUM_QUBITS * 3   # 150
N_CORES = 8
ROWS = BATCH // N_CORES          # 512 rows per core
KT = DIM // 128                  # 8 k-tiles of 128 along the feature dim
AT = DIM // 128                  # 8 amplitude tile-pairs (Re,Im) of 128

_BF16 = ml_dtypes.bfloat16
_CACHE = {}
LAST_RESULTS = None  # BassKernelResults of the most recent run (for test.py)


# ----------------------------------------------------------------------------
# Host: build the circuit unitary U (amp = U @ psi)
# ----------------------------------------------------------------------------
def _build_unitary(qw: np.ndarray) -> np.ndarray:
    qw = np.asarray(qw, dtype=np.float64)
    rotations = qw[:SIZE_ROT].reshape(LAYER_DEPTH + 1, NUM_QUBITS, 3)
    rxx = qw[SIZE_ROT:].reshape(LAYER_DEPTH, NUM_QUBITS - 1)

    # Columns of the identity, qubit axes unpacked: shape (2,)*10 + (DIM,)
    M = np.eye(DIM, dtype=np.complex128).reshape((2,) * NUM_QUBITS + (DIM,))

    def apply_r(M, theta, phi, alpha, j):
        sa = np.sin(alpha)
        nx = sa * np.cos(phi)
        ny = sa * np.sin(phi)
        nz = np.cos(alpha)
        ct = np.cos(theta)
        mist = -1j * np.sin(theta)
        U2 = np.array([
            [ct + mist * nz, mist * (nx - 1j * ny)],
            [mist * (nx + 1j * ny), ct - mist * nz],
        ], dtype=np.complex128)
        M = np.tensordot(U2, M, axes=[[1], [j]])
        return np.moveaxis(M, 0, j)

    for k in range(LAYER_DEPTH):
        for j in range(NUM_QUBITS):
            M = apply_r(M, rotations[k, j, 0], rotations[k, j, 1],
                        rotations[k, j, 2], j)
        for j in range(NUM_QUBITS - 1):
            flipped = np.flip(M, axis=(j, j + 1))
            M = np.cos(rxx[k, j]) * M + (-1j * np.sin(rxx[k, j])) * flipped
    for j in range(NUM_QUBITS):
        M = apply_r(M, rotations[LAYER_DEPTH, j, 0],
                    rotations[LAYER_DEPTH, j, 1],
                    rotations[LAYER_DEPTH, j, 2], j)
    return M.reshape(DIM, DIM)   # U with amp = U @ psi


# ----------------------------------------------------------------------------
# Device graph (built once, cached)
# ----------------------------------------------------------------------------
N_WARMUP_MM = 18  # dummy matmuls to lift the PE HAM throttle during DMA wait


def _build_graph():
    from concourse import bacc
    import concourse.mybir as mybir
    import concourse.tile as tile

    f32 = mybir.dt.float32
    bf16 = mybir.dt.bfloat16

    nc = bacc.Bacc("TRN2", target_bir_lowering=False, debug=False,
                   num_devices=N_CORES)

    # xt[p, k*ROWS + r] = X[r, 128k+p] (bf16)
    xt_d = nc.dram_tensor("xt", [128, KT, ROWS], bf16, kind="ExternalInput")
    # w[t, p, k*256 + j]: j<128 -> Ur[128t+j, 128k+p], j>=128 -> Ui[...]
    w_d = nc.dram_tensor("w", [AT, 128, KT, 256], bf16, kind="ExternalInput")
    # kc[p, t, o] = kernel[128t+p, o]
    kc_d = nc.dram_tensor("kc", [128, AT, NUM_OUTPUT], bf16, kind="ExternalInput")
    bias_d = nc.dram_tensor("bias", [NUM_OUTPUT, 1], f32, kind="ExternalInput")
    # transposed output; host does the final .T (tiny)
    out_d = nc.dram_tensor("out", [NUM_OUTPUT, ROWS], f32, kind="ExternalOutput")

    with tile.TileContext(nc) as tc:
        with (
            tc.tile_pool(name="xtp", bufs=1) as xtp,
            tc.tile_pool(name="wp", bufs=AT) as wp,
            tc.tile_pool(name="cst", bufs=1) as cst,
            tc.tile_pool(name="sq", bufs=2) as sqp,
            tc.tile_pool(name="pb", bufs=2) as pbp,
            tc.tile_pool(name="psmm", bufs=2, space="PSUM") as psmm,
            tc.tile_pool(name="psout", bufs=1, space="PSUM") as psout,
            tc.tile_pool(name="pswu", bufs=1, space="PSUM") as pswu,
        ):
            # PE warm-up: dummy matmuls on scratch data with no DMA deps, so
            # the HAM clock-gate lifts while input DMAs are still in flight.
            scratch = cst.tile([128, ROWS], bf16)
            nc.vector.memset(scratch[:], 0.0)
            wu_ps = pswu.tile([128, ROWS], f32)
            for i in range(N_WARMUP_MM):
                nc.tensor.matmul(wu_ps[:], scratch[:, 0:128], scratch[:],
                                 start=True, stop=True, skip_group_check=True)

            # Resident inputs. xt via HWDGE (sync); W slabs via SWDGE (gpsimd)
            # so the two paths stream concurrently.
            xt_sb = xtp.tile([128, KT, ROWS], bf16)
            nc.sync.dma_start(out=xt_sb[:], in_=xt_d[:])
            k_sb = cst.tile([128, AT, NUM_OUTPUT], bf16)
            nc.sync.dma_start(out=k_sb[:], in_=kc_d[:])
            bias_sb = cst.tile([NUM_OUTPUT, 1], f32)
            nc.sync.dma_start(out=bias_sb[:], in_=bias_d[:])
            w_slabs = []
            for t in range(AT):
                wt = wp.tile([128, KT, 256], bf16)
                nc.gpsimd.dma_start(out=wt[:], in_=w_d[t])
                w_slabs.append(wt)

            outT_ps = psout.tile([NUM_OUTPUT, ROWS], f32)

            for t in range(AT):
                wt = w_slabs[t]
                ps = psmm.tile([128, 2, ROWS], f32)  # [re|im], 2 PSUM banks
                for k in range(KT):
                    nc.tensor.matmul(ps[:, 0, :], wt[:, k, 0:128],
                                     xt_sb[:, k, :],
                                     start=(k == 0), stop=(k == KT - 1))
                for k in range(KT):
                    nc.tensor.matmul(ps[:, 1, :], wt[:, k, 128:256],
                                     xt_sb[:, k, :],
                                     start=(k == 0), stop=(k == KT - 1))
                sq = sqp.tile([128, 2, ROWS], f32)
                nc.scalar.square(sq[:], ps[:])
                p_t = pbp.tile([128, ROWS], bf16)
                nc.vector.tensor_add(p_t[:], sq[:, 0, :], sq[:, 1, :])
                nc.tensor.matmul(outT_ps[:], k_sb[:, t, :], p_t[:],
                                 start=(t == 0), stop=(t == AT - 1),
                                 skip_group_check=True)

            # Epilogue: bias add (per-partition) + contiguous store of out^T.
            outT_sb = cst.tile([NUM_OUTPUT, ROWS], f32)
            nc.scalar.activation(outT_sb[:], outT_ps[:],
                                 mybir.ActivationFunctionType.Identity,
                                 bias=bias_sb[:], scale=1.0)
            nc.sync.dma_start(out=out_d[:], in_=outT_sb[:])

    nc.compile()
    return nc


def _ensure_ntff_hook():
    """The trace path does `from antenv.axon_hooks import ...`; some images
    lack that optional module.  Provide it (wired to the axon PJRT .so when
    available) so BASS_TRACE=1 profiles instead of crashing."""
    try:
        import antenv.axon_hooks  # noqa: F401
        return
    except ImportError:
        pass
    import sys
    import types
    try:
        import antenv
    except ImportError:
        return
    mod = types.ModuleType("antenv.axon_hooks")
    state = {"hook": None}
    mod.set_axon_ntff_profile_hook = lambda h: state.__setitem__("hook", h)
    mod.get_axon_ntff_profile_hook = lambda: state["hook"]
    sys.modules["antenv.axon_hooks"] = mod
    antenv.axon_hooks = mod
    try:
        from trn_agent_boot.trn_boot import _ntff_profile_via_ctypes
        so_path = "/opt/axon/libaxon_pjrt.so"
        if os.path.exists(so_path):
            hook = _ntff_profile_via_ctypes(so_path)
            if hook is not None:
                mod.set_axon_ntff_profile_hook(hook)
    except Exception:
        pass


# ----------------------------------------------------------------------------
# Entry point
# ----------------------------------------------------------------------------
def kernel(x, quantum_weights, kernel, bias):
    global LAST_RESULTS
    _ensure_ntff_hook()
    from concourse.bass_utils import run_bass_kernel_spmd

    x = np.asarray(x, dtype=np.float32)
    qw = np.asarray(quantum_weights, dtype=np.float32)
    kmat = np.asarray(kernel, dtype=np.float32)
    bvec = np.asarray(bias, dtype=np.float32)

    U = _build_unitary(qw)
    # w[t, p, k, j]: j<128 -> Ur[128t+j, 128k+p]; j>=128 -> Ui[128t+j-128, 128k+p]
    Ur4 = U.real.reshape(AT, 128, KT, 128).transpose(0, 2, 3, 1)  # [t, k, p, j]
    Ui4 = U.imag.reshape(AT, 128, KT, 128).transpose(0, 2, 3, 1)
    w4 = np.concatenate([Ur4, Ui4], axis=3)                # [AT, KT, 128, 256]
    w4 = np.ascontiguousarray(w4.transpose(0, 2, 1, 3)).astype(_BF16)  # [t,p,k,j]

    kc = np.ascontiguousarray(
        kmat.reshape(AT, 128, NUM_OUTPUT).transpose(1, 0, 2)).astype(_BF16)
    bias2 = np.ascontiguousarray(bvec.reshape(NUM_OUTPUT, 1))

    if "nc" not in _CACHE:
        _CACHE["nc"] = _build_graph()
    nc = _CACHE["nc"]

    in_maps = []
    for c in range(N_CORES):
        xs = x[c * ROWS:(c + 1) * ROWS]                        # [512, 1024]
        # xt[p, k, r] = X[r, 128k+p]
        xt = np.ascontiguousarray(
            xs.T.reshape(KT, 128, ROWS).transpose(1, 0, 2)).astype(_BF16)
        in_maps.append({"xt": xt, "w": w4, "kc": kc, "bias": bias2})

    res = run_bass_kernel_spmd(nc, in_maps, core_ids=list(range(N_CORES)))
    LAST_RESULTS = res
    out = np.empty((BATCH, NUM_OUTPUT), dtype=np.float32)
    for c in range(N_CORES):
        out[c * ROWS:(c + 1) * ROWS] = res.results[c]["out"].T
    return out
